# revision 28
# baseline (speedup 1.0000x reference)
"""Trainium2 Bass kernel for nn_AttentionLayer_77558519431766.

Math: the reference computes softmax over a size-1 axis, which is
identically 1.0, so the attention MLP is dead code and

    out[b, e] = sum_{i<j} x[b,i,e] * x[b,j,e]
              = 0.5 * ((sum_f x[b,f,e])^2 - sum_f x[b,f,e]^2)

Design (v4, per 128-sample chunk, layout [128b, f*64+e]):
  - Input arrives via SWDGE cast-DMA (gpsimd): f32 in HBM -> bf16 in
    SBUF. Slices [24, 24, 2] fields per chunk: big slices stream, the
    tiny last slice keeps the post-stream serial tail short. DMA
    issues go first in the gpsimd program so the stream starts ASAP.
  - DVE computes s = sum_f x with an in-layout pairwise halving tree
    (5 big ops per 24-field slice; boundaries stay 64-col aligned).
  - PE transposes 128-col blocks (2 fields) into PSUM; ACT squares
    them back to SBUF with scale sqrt(0.5) (-> 0.5*x^2); PE runs a
    mask-stationary accumulation chain for q = sum_f 0.5*x^2
    (26.7ns/matmul warm - LDWEIGHTS fully pipelines).
  - res = 0.5*s^2 - q' via one ACT square + one DVE subtract.
  - Tail: drain only, no tile-emitted final barrier.

Sharding: pure data parallelism, batch 2048 -> 8 shards of 256.
"""

import numpy as np

try:
    import concourse.bass as bass  # noqa: F401
except ImportError:  # pragma: no cover
    import sys

    sys.path.insert(0, "/opt/trn_rl_repo")

_B, _F, _E = 2048, 50, 64
_NCORES = 8
_BS = _B // _NCORES  # 256 rows per core
_ROW = _F * _E  # 3200 floats per row
_P = 128  # SBUF partitions

_SLICES_C = {
    0: [(0, 768, 12), (768, 1536, 12), (1536, 3072, 24), (3072, 3200, 2)],
    1: [(0, 1536, 24), (1536, 3072, 24), (3072, 3200, 2)],
}
_TGROUPS = [(0, 5), (5, 10), (10, 15), (15, 20), (20, 25)]
_NBLK = 25


def _make_tc_class():
    """TileContext with a slim kernel tail: keep only the global-clock
    drain (output DMA completion)."""
    from concourse.tile import TileContext
    from concourse.vector_clock import ScopedClock

    class SlimTailTileContext(TileContext):
        def _drain_and_barrier(self, tick_clock, wait_clock):
            popped = self.nc._tile_sem_poison_stack.pop()
            assert popped is self._sem_poison

    return SlimTailTileContext


def _emit_tree(nc, xb, c0, nf, out_ap, sc):
    """Sum nf contiguous 64-col fields of xb starting at col c0 into
    out_ap [128, 64] via pairwise halving adds (bf16 temps in sc)."""
    add = nc.vector.tensor_add
    X = lambda a, b: xb[:, c0 + a : c0 + b]
    if nf == 2:
        add(out_ap, X(0, 64), X(64, 128))
        return
    if nf == 12:
        add(sc[:, 0:384], X(0, 384), X(384, 768))
        add(sc[:, 384:576], sc[:, 0:192], sc[:, 192:384])
        add(sc[:, 576:640], sc[:, 384:448], sc[:, 448:512])
        add(out_ap, sc[:, 576:640], sc[:, 512:576])
        return
    assert nf == 24
    add(sc[:, 0:768], X(0, 768), X(768, 1536))               # 12 fields
    add(sc[:, 768:1152], sc[:, 0:384], sc[:, 384:768])       # 6
    add(sc[:, 1152:1344], sc[:, 768:960], sc[:, 960:1152])   # 3
    add(sc[:, 1344:1408], sc[:, 1152:1216], sc[:, 1216:1280])  # 1 (+left)
    add(out_ap, sc[:, 1344:1408], sc[:, 1280:1344])


def _build():
    import concourse.bacc as bacc
    import concourse.mybir as mybir

    TileContext = _make_tc_class()

    f32 = mybir.dt.float32
    bf16 = mybir.dt.bfloat16
    i32 = mybir.dt.int32
    SQ = mybir.ActivationFunctionType.Square
    ALU = mybir.AluOpType
    HALF_SQRT = float(np.float32(np.sqrt(0.5)))

    nc = bacc.Bacc()
    x = nc.declare_dram_parameter("inputs", [_BS, _ROW], f32, isOutput=False)
    out = nc.declare_dram_parameter("out", [_BS, _E], f32, isOutput=True)

    n_chunks = _BS // _P  # 2

    with TileContext(nc) as tc:
        with (
            tc.tile_pool(name="consts", bufs=1) as cpool,
            tc.tile_pool(name="xb", bufs=2) as xbpool,
            tc.tile_pool(name="xsq", bufs=2) as sqpool,
            tc.tile_pool(name="tree", bufs=2) as trpool,
            tc.tile_pool(name="sp", bufs=2) as sppool,
            tc.tile_pool(name="pt", bufs=3, space="PSUM") as ptpool,
            tc.tile_pool(name="acc", bufs=2, space="PSUM") as accpool,
            tc.tile_pool(name="wp", bufs=1, space="PSUM") as wppool,
            tc.tile_pool(name="small", bufs=4) as spool,
        ):
            # ACT warm op first: hoists the ACT function-table load off
            # the critical path.
            warm = spool.tile([_P, 1], f32, tag="warm")
            nc.gpsimd.memset(warm[:], 0.0)
            nc.scalar.activation(warm[:], warm[:], SQ)

            # Tiles for both chunks.
            xbs, xsqs, sps, qts = [], [], [], []
            for c in range(n_chunks):
                xb = xbpool.tile([_P, _ROW], bf16, tag="xb")
                xsq = sqpool.tile([_P, _ROW], bf16, tag="xsq")
                sp = sppool.tile([_P, 4 * _E], f32, tag="sp")
                q_t = accpool.tile([_P, _E], f32, tag="q")
                xbs.append(xb)
                xsqs.append(xsq)
                sps.append(sp)
                qts.append(q_t)

            # Chunk 0's leading slices stream first...
            rows0 = slice(0, _P)
            for c0, c1, nf in _SLICES_C[0][:3]:
                nc.gpsimd.dma_start(out=xbs[0][:, c0:c1], in_=x[rows0, c0:c1])

            # PE warm-up source + two wide f32 dummy matmuls so the HAM
            # clock gate lifts to 2.4GHz before the real transposes.
            wsrc = cpool.tile([_P, 512], f32, tag="wsrc")
            nc.gpsimd.memset(wsrc[:], 0.0)
            wp = wppool.tile([_P, 512], f32, tag="wp")
            for _ in range(2):
                nc.tensor.matmul(wp[:], wsrc[:, 0:_P], wsrc[:], start=True, stop=True)

            # ...then constants (identity + stacked two-hot mask) while
            # the first slice is in flight...
            iot_i = cpool.tile([_P, _P], i32, tag="iot_i")
            iot_m = cpool.tile([_P, _E], i32, tag="iot_m")
            ident = cpool.tile([_P, _P], bf16, tag="ident")
            mask = cpool.tile([_P, _E], bf16, tag="mask")
            mask_b = cpool.tile([_P, _E], bf16, tag="mask_b")
            nc.gpsimd.iota(iot_i[:], pattern=[[1, _P]], base=0, channel_multiplier=-1)
            nc.gpsimd.iota(iot_m[:], pattern=[[1, _E]], base=0, channel_multiplier=-1)
            nc.vector.tensor_scalar(ident[:], iot_i[:], 0, None, op0=ALU.is_equal)
            nc.vector.tensor_scalar(mask[:], iot_m[:], 0, None, op0=ALU.is_equal)
            nc.vector.tensor_scalar(mask_b[:], iot_m[:], -_E, None, op0=ALU.is_equal)
            nc.vector.tensor_add(mask[:], mask[:], mask_b[:])

            # ...then the remaining input DMAs.
            rows1 = slice(_P, 2 * _P)
            nc.gpsimd.dma_start(
                out=xbs[0][:, 3072:3200], in_=x[rows0, 3072:3200]
            )
            for c0, c1, nf in _SLICES_C[1]:
                nc.gpsimd.dma_start(out=xbs[1][:, c0:c1], in_=x[rows1, c0:c1])

            # DVE s-trees per slice, both chunks; pre-add the two big
            # partials so the tail only needs one small add.
            p01s = []
            lasts = []
            for c in range(n_chunks):
                slcs = _SLICES_C[c]
                for si, (c0, c1, nf) in enumerate(slcs):
                    sc = trpool.tile([_P, 1408], bf16, tag="tr")
                    _emit_tree(
                        nc, xbs[c], c0, nf,
                        sps[c][:, si * _E : (si + 1) * _E], sc,
                    )
                acc = None
                for si in range(len(slcs) - 1):
                    if acc is None:
                        acc = sps[c][:, 0:64]
                        continue
                    nxt = spool.tile([_P, _E], f32, tag=f"pa_{c}_{si}")
                    nc.vector.tensor_add(
                        nxt[:], acc, sps[c][:, si * _E : (si + 1) * _E]
                    )
                    acc = nxt[:]
                p01s.append(acc)
                lasts.append((len(slcs) - 1) * _E)

            # PE transposes + ACT squares + q-chains + combine, software-
            # pipelined by one group so PE never stalls on ACT.
            for c in range(n_chunks):
                rows = slice(c * _P, (c + 1) * _P)
                xb, xsq, sparts = xbs[c], xsqs[c], sps[c]
                q_ps = qts[c][:]

                def emit_group_mms(g):
                    b0, b1 = _TGROUPS[g]
                    for k in range(b0, b1):
                        nc.tensor.matmul(
                            q_ps,
                            xsq[:, k * _P : (k + 1) * _P],
                            mask[:],
                            start=(k == 0),
                            stop=(k == _NBLK - 1),
                        )

                for g, (b0, b1) in enumerate(_TGROUPS):
                    gw = (b1 - b0) * _P
                    pt = ptpool.tile([_P, 5 * _P], bf16, tag="pt")
                    for j, k in enumerate(range(b0, b1)):
                        nc.tensor.transpose(
                            pt[:, j * _P : (j + 1) * _P],
                            xb[:, k * _P : (k + 1) * _P],
                            ident[:],
                        )
                    nc.scalar.activation(
                        xsq[:, b0 * _P : b1 * _P], pt[:, :gw], SQ, scale=HALF_SQRT
                    )
                    if g > 0:
                        emit_group_mms(g - 1)
                emit_group_mms(len(_TGROUPS) - 1)

                # s = p01 + p2; res = 0.5*s^2 - q'
                s_t = spool.tile([_P, _E], f32, tag=f"s_{c}")
                m2 = spool.tile([_P, _E], f32, tag=f"m2_{c}")
                res = spool.tile([_P, _E], f32, tag=f"res_{c}")
                nc.vector.tensor_add(s_t[:], p01s[c], sparts[:, lasts[c] : lasts[c] + _E])
                nc.scalar.activation(m2[:], s_t[:], SQ, scale=HALF_SQRT)
                nc.vector.tensor_sub(res[:], m2[:], q_ps)
                eng = nc.sync if c == 0 else nc.scalar
                eng.dma_start(out=out[rows, :], in_=res[:])
    nc.compile()
    return nc


_WALRUS_EXTRA = []


def _patch_walrus():
    """Cap walrus's semaphore allocation (unused semaphores cost ~150ns
    each in the NEFF postamble)."""
    from concourse import bass_utils

    if getattr(bass_utils, "_walrus_patched", False):
        return
    real_run = bass_utils.run_command

    def run2(cmd, **kw):
        if cmd and "walrus_driver" in str(cmd[0]):
            cmd = list(cmd) + _WALRUS_EXTRA
        return real_run(cmd, **kw)

    bass_utils.run_command = run2
    bass_utils._walrus_patched = True


def _run(in_maps, **kwargs):
    from concourse.bass_utils import run_bass_kernel_spmd

    _patch_walrus()
    nc = _build()
    return run_bass_kernel_spmd(nc, in_maps, core_ids=list(range(_NCORES)), **kwargs)


def _shard(inputs: np.ndarray):
    x = np.ascontiguousarray(
        np.asarray(inputs, dtype=np.float32).reshape(_B, _ROW)
    )
    return [
        {"inputs": np.ascontiguousarray(x[i * _BS : (i + 1) * _BS])}
        for i in range(_NCORES)
    ]


def kernel(
    inputs: np.ndarray,
    weight_attention: np.ndarray = None,
    weight_projection: np.ndarray = None,
    weight_bias: np.ndarray = None,
) -> np.ndarray:
    # weights are dead code (softmax over a size-1 axis == 1.0)
    res = _run(_shard(inputs))
    return np.concatenate([r["out"] for r in res.results], axis=0)
